# revision 11
# baseline (speedup 1.0000x reference)
"""Trainium2 Bass kernel for the char-LSTM word-similarity CNN scorer.

Problem: B=8192 examples x NW=4 words x L=16 chars. Per word: char
embeddings -> masked LSTMCell over <=16 steps -> cell state c [128].
Per example: 4x4 cosine matrix of the word reps -> 2-layer 2x2-valid
CNN -> linear scorer -> sigmoid.

Strategy (pure data parallel, 1024 examples / 4096 words per core):
 - Host folds emb @ W_ih.T + (b_ih + b_hh) into a [65, 512] table G65;
   per-step char inputs become a K=65 one-hot matmul (row 64 = "freeze"
   flag that drives f->1, i->0 for words past their length, so no
   masking/select ops are needed on device).
 - Words are sorted by length (desc) on host; step t only processes
   ceil(max_core_Nt/512) blocks of 512 words. State lives as
   [H=128 partitions, word] so the recurrent matmul is weights-stationary
   with zero transposes.
 - Tail: transpose c to [word, H], l2-normalize rows via ln/exp,
   round-trip through DRAM with an indirect-DMA gather to undo the sort
   and group words per example, cosine = fused mul+reduce per word pair,
   then the 2x2 convs + scorer lowered to tiny host-built matmuls.
"""

import os
import sys

for _p in ("/opt/trn_rl_repo",):
    if _p not in sys.path and os.path.isdir(_p):
        sys.path.insert(0, _p)

import numpy as np

import concourse.bass as bass
import concourse.mybir as mybir
import concourse.tile as tile
from concourse.bass_utils import run_bass_kernel_spmd
from concourse.masks import make_identity

# This container's walrus build rejects CTRL instructions (Drain) carrying
# more than 2 sync waits ("Too many sync wait commands" in setupSyncWait).
# Tile's kernel-tail drain accumulates one wait per engine/DMA-queue sem, so
# redistribute: keep one wait on the drain, move the rest onto nofuse NOPs
# that execute before the all-engine barrier. Semantics are unchanged (all
# waits still complete before the barrier / semaphore teardown).
def _patched_drain_and_barrier(self, tick_clock, wait_clock):
    nc = self.nc
    drain_inst = nc.sync.drain()
    wait_clock.add_sem_waits(
        drain_inst.ins, tile.ScopedClock({None: tick_clock.global_clock})
    )
    waits = list(drain_inst.ins.sync_info.on_wait)
    if len(waits) > 1:
        drain_inst.ins.sync_info.on_wait = waits[:1]
        for k in range(1, len(waits)):
            nop = nc.sync.nop(nofuse=True, hint="drain_wait_spill")
            if nop.ins.sync_info is None:
                nop.ins.sync_info = mybir.SyncInfo(on_wait=[], on_update=[])
            nop.ins.sync_info.on_wait = [waits[k]]
    nc.all_engine_barrier()
    assert self.sems is not None
    popped = nc._tile_sem_poison_stack.pop()
    assert popped is self._sem_poison
    nc.clear_and_free_semaphores(list(self.sems.allocated().values()))
    nc.all_engine_barrier()


tile.TileContext._drain_and_barrier = _patched_drain_and_barrier

def _spill_excess_waits(nc):
    """Walrus here rejects instructions with more than ~2 sync waits. Spill
    excess waits onto same-engine NoOps inserted just before the instruction
    (engines dispatch in program order, so waiting earlier on the same engine
    is equivalent)."""
    cnt = [0]
    for fn in nc.m.functions:
        for bb in fn.blocks:
            insts = list(bb.instructions)
            out = []
            changed = False
            for inst in insts:
                si = inst.sync_info
                waits = list(si.on_wait) if si is not None and si.on_wait else []
                max_waits = 1
                if len(waits) > max_waits:
                    changed = True
                    keep = waits[-max_waits:]
                    extra = waits[:-max_waits]
                    for j in range(0, len(extra), max_waits):
                        cnt[0] += 1
                        nop = mybir.InstNoOp(name=f"I-spillw-{cnt[0]}", ins=[], outs=[])
                        nop.engine = inst.engine
                        nop.sync_info = mybir.SyncInfo(
                            on_wait=extra[j:j + max_waits], on_update=[])
                        nop.bass_nofuse = True
                        nop.bass_priority = 0
                        nop.text_hint = "spillw"
                        nop.debug = inst.debug
                        out.append(nop)
                    si.on_wait = keep
                out.append(inst)
            if changed:
                bb.instructions = out

B, NW, L, E, H, V = 8192, 4, 16, 128, 128, 64
NCORES = 8
PER = B // NCORES          # 1024 examples per core
NWORD = PER * NW           # 4096 words per core
NBLK = NWORD // 512        # 8 blocks of 512 words
NEC = PER // 128           # 8 example-chunks of 128
BLK = 512
FB = 30.0                  # freeze bias magnitude
F32 = mybir.dt.float32
AF = mybir.ActivationFunctionType
ALU = mybir.AluOpType

P6 = [(0, 1), (0, 2), (0, 3), (1, 2), (1, 3), (2, 3)]


# ----------------------------------------------------------------- host prep

def _build_consts(inp):
    emb = np.asarray(inp["emb_i"], np.float32)
    W_ih = np.asarray(inp["W_ih"], np.float32)
    W_hh = np.asarray(inp["W_hh"], np.float32)
    b = np.asarray(inp["b_ih"], np.float32) + np.asarray(inp["b_hh"], np.float32)
    G65 = np.zeros((V + 1, 4 * H), np.float32)
    G65[:V] = emb @ W_ih.T + b
    G65[V, 0:H] = -FB
    G65[V, H:2 * H] = +FB
    WhhT = np.ascontiguousarray(W_hh.T)

    w1 = np.asarray(inp["conv1_w"], np.float32)
    b1 = np.asarray(inp["conv1_b"], np.float32)
    w2 = np.asarray(inp["conv2_w"], np.float32)
    b2 = np.asarray(inp["conv2_b"], np.float32)
    ws = np.asarray(inp["scorer_w"], np.float32)
    bs = float(np.asarray(inp["scorer_b"], np.float32)[0])

    p6idx = {p: i for i, p in enumerate(P6)}
    W1eff = np.zeros((6, 36), np.float32)
    b1eff = np.zeros((36, 1), np.float32)
    for c in range(4):
        for y in range(3):
            for x in range(3):
                m = c * 9 + y * 3 + x
                b1eff[m, 0] += b1[c]
                for dy in range(2):
                    for dx in range(2):
                        a, bb = y + dy, x + dx
                        w = w1[c, 0, dy, dx]
                        if a == bb:
                            b1eff[m, 0] += w
                        else:
                            W1eff[p6idx[(min(a, bb), max(a, bb))], m] += w
    W2eff = np.zeros((36, 32), np.float32)
    b2eff = np.zeros((32, 1), np.float32)
    for c2 in range(8):
        for y in range(2):
            for x in range(2):
                m = c2 * 4 + y * 2 + x
                b2eff[m, 0] = b2[c2]
                for c1 in range(4):
                    for dy in range(2):
                        for dx in range(2):
                            W2eff[c1 * 9 + (y + dy) * 3 + (x + dx), m] += w2[c2, c1, dy, dx]
    Wsc = ws[0].astype(np.float32).reshape(32, 1)
    return dict(G65=G65, WhhT=WhhT, W1eff=W1eff, b1eff=b1eff,
                W2eff=W2eff, b2eff=b2eff, Wsc=Wsc, bsc=bs)


def _core_prep(word_ids_c, lengths_c):
    wid = np.asarray(word_ids_c).reshape(NWORD, L)
    lens = np.asarray(lengths_c).reshape(NWORD)
    perm = np.argsort(-lens, kind="stable")
    inv = np.empty(NWORD, np.int32)
    inv[perm] = np.arange(NWORD, dtype=np.int32)
    wid_s = wid[perm]
    lens_s = lens[perm]
    Nt = (np.arange(L)[:, None] < lens_s[None, :]).sum(1)  # [L]
    return wid_s, lens_s, Nt, inv


def _build_onehot(wid_s, lens_s, widths):
    oh = np.zeros((L, V + 1, NWORD), np.float32)
    cols = np.arange(NWORD)
    for t in range(L):
        n = int(widths[t])
        if n == 0:
            continue
        alive = lens_s[:n] > t
        oh[t, wid_s[:n, t], cols[:n]] = alive.astype(np.float32)
        oh[t, V, cols[:n]] = (~alive).astype(np.float32)
    return oh


def _build_idx(inv):
    # idx[p, i*NEC + ec] = sorted-position of original word 4*(ec*128+p)+i
    idx = np.empty((128, NW * NEC), np.int32)
    p = np.arange(128)
    for i in range(NW):
        for ec in range(NEC):
            idx[:, i * NEC + ec] = inv[NW * (ec * 128 + p) + i]
    return idx


# -------------------------------------------------------------- bass program

def _build_program(bt):
    """bt: tuple of per-step block counts (len L, each 0..8)."""
    nc = bass.Bass()

    oh_in = nc.dram_tensor("oh", [L, V + 1, NWORD], F32, kind="ExternalInput")
    idx_in = nc.dram_tensor("idx", [128, NW * NEC], mybir.dt.int32, kind="ExternalInput")
    g65_in = nc.dram_tensor("g65", [V + 1, 4 * H], F32, kind="ExternalInput")
    whht_in = nc.dram_tensor("whht", [H, 4 * H], F32, kind="ExternalInput")
    w1_in = nc.dram_tensor("w1", [6, 36], F32, kind="ExternalInput")
    b1_in = nc.dram_tensor("b1", [36, 1], F32, kind="ExternalInput")
    w2_in = nc.dram_tensor("w2", [36, 32], F32, kind="ExternalInput")
    b2_in = nc.dram_tensor("b2", [32, 1], F32, kind="ExternalInput")
    wsc_in = nc.dram_tensor("wsc", [32, 1], F32, kind="ExternalInput")
    bsc_in = nc.dram_tensor("bsc", [1, 1], F32, kind="ExternalInput")
    out_d = nc.dram_tensor("out", [1, PER], F32, kind="ExternalOutput")
    c_dram = nc.dram_tensor("cscratch", [NWORD, H], F32)

    with tile.TileContext(nc) as tc:
        with (
            tc.tile_pool(name="const", bufs=1) as cpool,
            tc.tile_pool(name="state", bufs=1) as spool,
        ):
            g65_sb = cpool.tile([V + 1, 4 * H], F32, tag="g65", name="g65")
            whht_sb = cpool.tile([H, 4 * H], F32, tag="whht", name="whht")
            w1_sb = cpool.tile([6, 36], F32, tag="w1", name="w1")
            b1_sb = cpool.tile([36, 1], F32, tag="b1", name="b1")
            w2_sb = cpool.tile([36, 32], F32, tag="w2", name="w2")
            b2_sb = cpool.tile([32, 1], F32, tag="b2", name="b2")
            wsc_sb = cpool.tile([32, 1], F32, tag="wsc", name="wsc")
            bsc_sb = cpool.tile([1, 1], F32, tag="bsc", name="bsc")
            ident = cpool.tile([128, 128], F32, tag="ident", name="ident")
            idx_sb = cpool.tile([128, NW * NEC], mybir.dt.int32, tag="idx", name="idx")
            for sb, dr in ((g65_sb, g65_in), (whht_sb, whht_in), (w1_sb, w1_in),
                           (b1_sb, b1_in), (w2_sb, w2_in), (b2_sb, b2_in),
                           (wsc_sb, wsc_in), (bsc_sb, bsc_in), (idx_sb, idx_in)):
                nc.sync.dma_start(sb[:], dr[:])
            make_identity(nc, ident[:])

            h_t = [spool.tile([H, BLK], F32, tag=f"h{k}", name=f"h{k}") for k in range(NBLK)]
            c_t = [spool.tile([H, BLK], F32, tag=f"c{k}", name=f"c{k}") for k in range(NBLK)]
            for k in range(NBLK):
                nc.gpsimd.memset(c_t[k][:], 0.0)
                if bt[0] <= k:  # never matmul'd -> h must still be defined
                    nc.gpsimd.memset(h_t[k][:], 0.0)

            # ------------------------------------------------ LSTM main loop
            with (
                tc.tile_pool(name="oh", bufs=2) as ohpool,
                tc.tile_pool(name="gpsum", bufs=2, space="PSUM") as gpsum,
                tc.tile_pool(name="gsb", bufs=3) as gsb,
            ):
                for t in range(L):
                    nb = bt[t]
                    if nb == 0:
                        continue
                    w = nb * BLK
                    oh_sb = ohpool.tile([V + 1, NWORD], F32, tag="oh", name="oh")
                    nc.sync.dma_start(oh_sb[:, :w], oh_in[t, :, :w])
                    last = t == L - 1
                    for k in range(nb):
                        ps = [gpsum.tile([128, BLK], F32, tag=f"p{m}", name=f"p{m}") for m in range(4)]
                        ohk = oh_sb[:, k * BLK:(k + 1) * BLK]
                        for m in range(4):
                            sl = slice(m * H, (m + 1) * H)
                            if t == 0:
                                nc.tensor.matmul(ps[m][:], lhsT=g65_sb[:, sl],
                                                 rhs=ohk, start=True, stop=True)
                            else:
                                nc.tensor.matmul(ps[m][:], lhsT=g65_sb[:, sl],
                                                 rhs=ohk, start=True, stop=False)
                                nc.tensor.matmul(ps[m][:], lhsT=whht_sb[:, sl],
                                                 rhs=h_t[k][:], start=False, stop=True)
                        ti = gsb.tile([128, BLK], F32, tag="ti", name="ti")
                        tf = gsb.tile([128, BLK], F32, tag="tf", name="tf")
                        tg = gsb.tile([128, BLK], F32, tag="tg", name="tg")
                        nc.scalar.activation(ti[:], ps[0][:], AF.Sigmoid)
                        nc.scalar.activation(tf[:], ps[1][:], AF.Sigmoid)
                        nc.scalar.activation(tg[:], ps[2][:], AF.Tanh)
                        nc.vector.tensor_mul(tg[:], ti[:], tg[:])      # i*g
                        nc.vector.tensor_mul(c_t[k][:], tf[:], c_t[k][:])
                        nc.vector.tensor_add(c_t[k][:], c_t[k][:], tg[:])
                        if not last:
                            to = gsb.tile([128, BLK], F32, tag="to", name="to")
                            tt = gsb.tile([128, BLK], F32, tag="tt", name="tt")
                            nc.scalar.activation(to[:], ps[3][:], AF.Sigmoid)
                            nc.scalar.activation(tt[:], c_t[k][:], AF.Tanh)
                            nc.vector.tensor_mul(h_t[k][:], to[:], tt[:])

            # ------------------------------------------------------- tail
            with (
                tc.tile_pool(name="big", bufs=1) as big,
                tc.tile_pool(name="tpsum", bufs=2, space="PSUM") as tpsum,
                tc.tile_pool(name="cpsum", bufs=2, space="PSUM") as cpsum,
                tc.tile_pool(name="small", bufs=1) as small,
                tc.tile_pool(name="scr", bufs=2) as scrp,
            ):
                cT = big.tile([128, NWORD], F32, tag="cT", name="cT")
                A = big.tile([128, NWORD], F32, tag="A", name="A")
                d_all = small.tile([128, 32], F32, tag="d", name="d")
                s_all = small.tile([128, 32], F32, tag="s", name="s")
                D_all = small.tile([128, NEC * 6], F32, tag="D", name="D")
                cos6 = small.tile([6, PER], F32, tag="cos6", name="cos6")

                for k in range(32):
                    pt = tpsum.tile([128, 128], F32, tag="tp", name="tp")
                    src = c_t[k // 4][:, (k % 4) * 128:(k % 4 + 1) * 128]
                    nc.tensor.transpose(pt[:], src, ident[:])
                    cslc = cT[:, k * 128:(k + 1) * 128]
                    nc.vector.tensor_copy(cslc, pt[:])
                    scr = scrp.tile([128, 128], F32, tag="scr", name="scr")
                    nc.vector.tensor_mul(scr[:], cslc, cslc)
                    nc.vector.tensor_reduce(
                        d_all[:, k:k + 1], scr[:], axis=mybir.AxisListType.X,
                        op=ALU.add)
                nc.vector.tensor_scalar_max(d_all[:], d_all[:], 1e-30)
                nc.scalar.activation(s_all[:], d_all[:], AF.Ln)
                nc.scalar.activation(s_all[:], s_all[:], AF.Exp, scale=-0.5)
                for k in range(32):
                    cslc = cT[:, k * 128:(k + 1) * 128]
                    nc.vector.tensor_scalar_mul(cslc, cslc, s_all[:, k:k + 1])

                nc.sync.dma_start(
                    c_dram[:].rearrange("(k p) h -> p k h", p=128),
                    cT[:].rearrange("p (k h) -> p k h", k=32))

                for b in range(NW * NEC):
                    nc.gpsimd.indirect_dma_start(
                        out=A[:, b * 128:(b + 1) * 128],
                        out_offset=None,
                        in_=c_dram[:],
                        in_offset=bass.IndirectOffsetOnAxis(ap=idx_sb[:, b:b + 1], axis=0),
                    )

                for ec in range(NEC):
                    for k, (i, j) in enumerate(P6):
                        scr = scrp.tile([128, 128], F32, tag="scr", name="scr")
                        nc.vector.tensor_mul(
                            scr[:],
                            A[:, (i * NEC + ec) * 128:(i * NEC + ec + 1) * 128],
                            A[:, (j * NEC + ec) * 128:(j * NEC + ec + 1) * 128])
                        nc.vector.tensor_reduce(
                            D_all[:, ec * 6 + k:ec * 6 + k + 1], scr[:],
                            axis=mybir.AxisListType.X, op=ALU.add)
                for ec in range(NEC):
                    pt = tpsum.tile([128, 128], F32, tag="tp", name="tp")
                    nc.tensor.transpose(pt[:6, :], D_all[:, ec * 6:ec * 6 + 6], ident[:])
                    nc.vector.tensor_copy(cos6[:, ec * 128:(ec + 1) * 128], pt[:6, :])

                r1 = small.tile([36, PER], F32, tag="r1", name="r1")
                r2 = small.tile([32, PER], F32, tag="r2", name="r2")
                e_sb = small.tile([1, PER], F32, tag="e", name="e")
                o_sb = small.tile([1, PER], F32, tag="o", name="o")
                p1 = cpsum.tile([36, PER], F32, tag="cp1", name="cp1")
                for half in range(2):
                    sl = slice(half * 512, (half + 1) * 512)
                    nc.tensor.matmul(p1[:, sl], lhsT=w1_sb[:], rhs=cos6[:, sl],
                                     start=True, stop=True)
                nc.scalar.activation(r1[:], p1[:], AF.Relu, bias=b1_sb[:, 0:1])
                p2 = cpsum.tile([32, PER], F32, tag="cp1", name="cp1")
                for half in range(2):
                    sl = slice(half * 512, (half + 1) * 512)
                    nc.tensor.matmul(p2[:, sl], lhsT=w2_sb[:], rhs=r1[:, sl],
                                     start=True, stop=True)
                nc.scalar.activation(r2[:], p2[:], AF.Relu, bias=b2_sb[:, 0:1])
                p3 = cpsum.tile([1, PER], F32, tag="cp1", name="cp1")
                for half in range(2):
                    sl = slice(half * 512, (half + 1) * 512)
                    nc.tensor.matmul(p3[:, sl], lhsT=wsc_sb[:], rhs=r2[:, sl],
                                     start=True, stop=True)
                # sigmoid(x + bsc) = 1 / (1 + exp(-x - bsc)), staying in the
                # exp table set (avoids a 2.7us table switch for one op)
                nbsc = small.tile([1, 1], F32, tag="nbsc", name="nbsc")
                nc.vector.tensor_scalar_mul(nbsc[:], bsc_sb[:], -1.0)
                nc.scalar.activation(e_sb[:], p3[:], AF.Exp, scale=-1.0,
                                     bias=nbsc[:, 0:1])
                nc.vector.tensor_scalar_add(e_sb[:], e_sb[:], 1.0)
                nc.vector.reciprocal(o_sb[:], e_sb[:])
                nc.sync.dma_start(out_d[:], o_sb[:])

    return nc


_prog_cache = {}


def _get_program(bt):
    key = tuple(int(x) for x in bt)
    if key not in _prog_cache:
        _prog_cache[key] = _build_program(key)
    return _prog_cache[key]


def _run(inputs, trace=False):
    consts = _build_consts(inputs)
    word_ids = np.asarray(inputs["word_ids"])
    lengths = np.asarray(inputs["lengths"])

    preps = []
    for c in range(NCORES):
        sl = slice(c * PER, (c + 1) * PER)
        preps.append(_core_prep(word_ids[sl], lengths[sl]))
    Nt_max = np.stack([p[2] for p in preps]).max(0)
    bt = tuple(int(x) for x in np.ceil(Nt_max / BLK).astype(np.int64))
    widths = [b * BLK for b in bt]

    in_maps = []
    for c in range(NCORES):
        wid_s, lens_s, _, inv = preps[c]
        in_maps.append({
            "oh": _build_onehot(wid_s, lens_s, widths),
            "idx": _build_idx(inv),
            "g65": consts["G65"], "whht": consts["WhhT"],
            "w1": consts["W1eff"], "b1": consts["b1eff"],
            "w2": consts["W2eff"], "b2": consts["b2eff"],
            "wsc": consts["Wsc"],
            "bsc": np.full((1, 1), consts["bsc"], np.float32),
        })

    nc = _get_program(bt)
    _spill_excess_waits(nc)  # idempotent; HW-compile only (CoreSim dislikes raw NoOps)
    res = run_bass_kernel_spmd(nc, in_maps, list(range(NCORES)), trace=trace)
    out = np.concatenate([np.asarray(r["out"]).reshape(PER) for r in res.results])
    return out.reshape(B, 1).astype(np.float32), res.exec_time_ns


def kernel(**inputs):
    return _run(inputs)[0]


# revision 13
# speedup vs baseline: 2.2305x; 2.2305x over previous
"""Trainium2 Bass kernel for the char-LSTM word-similarity CNN scorer.

Problem: B=8192 examples x NW=4 words x L=16 chars. Per word: char
embeddings -> masked LSTMCell over <=16 steps -> cell state c [128].
Per example: 4x4 cosine matrix of the word reps -> 2-layer 2x2-valid
CNN -> linear scorer -> sigmoid.

Strategy (pure data parallel, 1024 examples / 4096 words per core):
 - Host folds emb @ W_ih.T + (b_ih + b_hh) into a [65, 512] table G65;
   per-step char inputs become a K=65 one-hot matmul (row 64 = "freeze"
   flag that drives f->1, i->0 for words past their length, so no
   masking/select ops are needed on device).
 - Words are sorted by length (desc) on host; step t only processes
   ceil(max_core_Nt/512) blocks of 512 words. State lives as
   [H=128 partitions, word] so the recurrent matmul is weights-stationary
   with zero transposes.
 - Tail: transpose c to [word, H], l2-normalize rows via ln/exp,
   round-trip through DRAM with an indirect-DMA gather to undo the sort
   and group words per example, cosine = fused mul+reduce per word pair,
   then the 2x2 convs + scorer lowered to tiny host-built matmuls.
"""

import os
import sys

for _p in ("/opt/trn_rl_repo",):
    if _p not in sys.path and os.path.isdir(_p):
        sys.path.insert(0, _p)

import ml_dtypes
import numpy as np

import concourse.bass as bass
import concourse.mybir as mybir
import concourse.tile as tile
from concourse.bass_utils import run_bass_kernel_spmd
from concourse.masks import make_identity

# This container's walrus build rejects CTRL instructions (Drain) carrying
# more than 2 sync waits ("Too many sync wait commands" in setupSyncWait).
# Tile's kernel-tail drain accumulates one wait per engine/DMA-queue sem, so
# redistribute: keep one wait on the drain, move the rest onto nofuse NOPs
# that execute before the all-engine barrier. Semantics are unchanged (all
# waits still complete before the barrier / semaphore teardown).
def _patched_drain_and_barrier(self, tick_clock, wait_clock):
    nc = self.nc
    drain_inst = nc.sync.drain()
    wait_clock.add_sem_waits(
        drain_inst.ins, tile.ScopedClock({None: tick_clock.global_clock})
    )
    waits = list(drain_inst.ins.sync_info.on_wait)
    if len(waits) > 1:
        drain_inst.ins.sync_info.on_wait = waits[:1]
        for k in range(1, len(waits)):
            nop = nc.sync.nop(nofuse=True, hint="drain_wait_spill")
            if nop.ins.sync_info is None:
                nop.ins.sync_info = mybir.SyncInfo(on_wait=[], on_update=[])
            nop.ins.sync_info.on_wait = [waits[k]]
    nc.all_engine_barrier()
    assert self.sems is not None
    popped = nc._tile_sem_poison_stack.pop()
    assert popped is self._sem_poison
    nc.clear_and_free_semaphores(list(self.sems.allocated().values()))
    nc.all_engine_barrier()


tile.TileContext._drain_and_barrier = _patched_drain_and_barrier

def _spill_excess_waits(nc):
    """Walrus here rejects instructions with more than ~2 sync waits. Spill
    excess waits onto same-engine NoOps inserted just before the instruction
    (engines dispatch in program order, so waiting earlier on the same engine
    is equivalent)."""
    cnt = [0]
    for fn in nc.m.functions:
        for bb in fn.blocks:
            insts = list(bb.instructions)
            out = []
            changed = False
            for inst in insts:
                si = inst.sync_info
                waits = list(si.on_wait) if si is not None and si.on_wait else []
                max_waits = 1
                if len(waits) > max_waits:
                    changed = True
                    keep = waits[-max_waits:]
                    extra = waits[:-max_waits]
                    for j in range(0, len(extra), max_waits):
                        cnt[0] += 1
                        nop = mybir.InstNoOp(name=f"I-spillw-{cnt[0]}", ins=[], outs=[])
                        nop.engine = inst.engine
                        nop.sync_info = mybir.SyncInfo(
                            on_wait=extra[j:j + max_waits], on_update=[])
                        nop.bass_nofuse = True
                        nop.bass_priority = 0
                        nop.text_hint = "spillw"
                        nop.debug = inst.debug
                        out.append(nop)
                    si.on_wait = keep
                out.append(inst)
            if changed:
                bb.instructions = out

B, NW, L, E, H, V = 8192, 4, 16, 128, 128, 64
NCORES = 8
PER = B // NCORES          # 1024 examples per core
NWORD = PER * NW           # 4096 words per core
NBLK = NWORD // 512        # 8 blocks of 512 words
NEC = PER // 128           # 8 example-chunks of 128
BLK = 512
FB = 30.0                  # freeze bias magnitude
F32 = mybir.dt.float32
BF16 = mybir.dt.bfloat16
AF = mybir.ActivationFunctionType
ALU = mybir.AluOpType

P6 = [(0, 1), (0, 2), (0, 3), (1, 2), (1, 3), (2, 3)]


# ----------------------------------------------------------------- host prep

def _build_consts(inp):
    emb = np.asarray(inp["emb_i"], np.float32)
    W_ih = np.asarray(inp["W_ih"], np.float32)
    W_hh = np.asarray(inp["W_hh"], np.float32)
    b = np.asarray(inp["b_ih"], np.float32) + np.asarray(inp["b_hh"], np.float32)
    G65 = np.zeros((V + 2, 4 * H), np.float32)
    G65[:V] = emb @ W_ih.T + b
    G65[V, 0:H] = -FB
    G65[V, H:2 * H] = +FB
    WhhT = np.ascontiguousarray(W_hh.T)

    w1 = np.asarray(inp["conv1_w"], np.float32)
    b1 = np.asarray(inp["conv1_b"], np.float32)
    w2 = np.asarray(inp["conv2_w"], np.float32)
    b2 = np.asarray(inp["conv2_b"], np.float32)
    ws = np.asarray(inp["scorer_w"], np.float32)
    bs = float(np.asarray(inp["scorer_b"], np.float32)[0])

    p6idx = {p: i for i, p in enumerate(P6)}
    W1eff = np.zeros((6, 36), np.float32)
    b1eff = np.zeros((36, 1), np.float32)
    for c in range(4):
        for y in range(3):
            for x in range(3):
                m = c * 9 + y * 3 + x
                b1eff[m, 0] += b1[c]
                for dy in range(2):
                    for dx in range(2):
                        a, bb = y + dy, x + dx
                        w = w1[c, 0, dy, dx]
                        if a == bb:
                            b1eff[m, 0] += w
                        else:
                            W1eff[p6idx[(min(a, bb), max(a, bb))], m] += w
    W2eff = np.zeros((36, 32), np.float32)
    b2eff = np.zeros((32, 1), np.float32)
    for c2 in range(8):
        for y in range(2):
            for x in range(2):
                m = c2 * 4 + y * 2 + x
                b2eff[m, 0] = b2[c2]
                for c1 in range(4):
                    for dy in range(2):
                        for dx in range(2):
                            W2eff[c1 * 9 + (y + dy) * 3 + (x + dx), m] += w2[c2, c1, dy, dx]
    Wsc = ws[0].astype(np.float32).reshape(32, 1)
    return dict(G65=G65, WhhT=WhhT, W1eff=W1eff, b1eff=b1eff,
                W2eff=W2eff, b2eff=b2eff, Wsc=Wsc, bsc=bs)


def _core_prep(word_ids_c, lengths_c):
    wid = np.asarray(word_ids_c).reshape(NWORD, L)
    lens = np.asarray(lengths_c).reshape(NWORD)
    perm = np.argsort(-lens, kind="stable")
    inv = np.empty(NWORD, np.int32)
    inv[perm] = np.arange(NWORD, dtype=np.int32)
    wid_s = wid[perm]
    lens_s = lens[perm]
    Nt = (np.arange(L)[:, None] < lens_s[None, :]).sum(1)  # [L]
    return wid_s, lens_s, Nt, inv


def _build_onehot(wid_s, lens_s, widths):
    oh = np.zeros((L, V + 2, NWORD), np.float32)
    cols = np.arange(NWORD)
    for t in range(L):
        n = int(widths[t])
        if n == 0:
            continue
        alive = lens_s[:n] > t
        oh[t, wid_s[:n, t], cols[:n]] = alive.astype(np.float32)
        oh[t, V, cols[:n]] = (~alive).astype(np.float32)
    return oh


def _build_idx(inv):
    # idx[p, i*NEC + ec] = sorted-position of original word 4*(ec*128+p)+i
    idx = np.empty((128, NW * NEC), np.int32)
    p = np.arange(128)
    for i in range(NW):
        for ec in range(NEC):
            idx[:, i * NEC + ec] = inv[NW * (ec * 128 + p) + i]
    return idx


# -------------------------------------------------------------- bass program

def _build_program(bt):
    """bt: tuple of per-step block counts (len L, each 0..8)."""
    nc = bass.Bass()

    oh_in = nc.dram_tensor("oh", [L, V + 2, NWORD], BF16, kind="ExternalInput")
    idx_in = nc.dram_tensor("idx", [128, NW * NEC], mybir.dt.int32, kind="ExternalInput")
    g65_in = nc.dram_tensor("g65", [V + 2, 4 * H], BF16, kind="ExternalInput")
    whht_in = nc.dram_tensor("whht", [H, 4 * H], BF16, kind="ExternalInput")
    w1_in = nc.dram_tensor("w1", [6, 36], F32, kind="ExternalInput")
    b1_in = nc.dram_tensor("b1", [36, 1], F32, kind="ExternalInput")
    w2_in = nc.dram_tensor("w2", [36, 32], F32, kind="ExternalInput")
    b2_in = nc.dram_tensor("b2", [32, 1], F32, kind="ExternalInput")
    wsc_in = nc.dram_tensor("wsc", [32, 1], F32, kind="ExternalInput")
    bsc_in = nc.dram_tensor("bsc", [1, 1], F32, kind="ExternalInput")
    out_d = nc.dram_tensor("out", [1, PER], F32, kind="ExternalOutput")
    c_dram = nc.dram_tensor("cscratch", [NWORD, H], F32)

    with tile.TileContext(nc) as tc:
        with (
            tc.tile_pool(name="const", bufs=1) as cpool,
            tc.tile_pool(name="state", bufs=1) as spool,
        ):
            g65_sb = cpool.tile([V + 2, 4 * H], BF16, tag="g65", name="g65")
            whht_sb = cpool.tile([H, 4 * H], BF16, tag="whht", name="whht")
            w1_sb = cpool.tile([6, 36], F32, tag="w1", name="w1")
            b1_sb = cpool.tile([36, 1], F32, tag="b1", name="b1")
            w2_sb = cpool.tile([36, 32], F32, tag="w2", name="w2")
            b2_sb = cpool.tile([32, 1], F32, tag="b2", name="b2")
            wsc_sb = cpool.tile([32, 1], F32, tag="wsc", name="wsc")
            bsc_sb = cpool.tile([1, 1], F32, tag="bsc", name="bsc")
            ident = cpool.tile([128, 128], F32, tag="ident", name="ident")
            idx_sb = cpool.tile([128, NW * NEC], mybir.dt.int32, tag="idx", name="idx")
            for sb, dr in ((g65_sb, g65_in), (whht_sb, whht_in), (w1_sb, w1_in),
                           (b1_sb, b1_in), (w2_sb, w2_in), (b2_sb, b2_in),
                           (wsc_sb, wsc_in), (bsc_sb, bsc_in), (idx_sb, idx_in)):
                nc.sync.dma_start(sb[:], dr[:])
            make_identity(nc, ident[:])

            h_t = [spool.tile([H, BLK], BF16, tag=f"h{k}", name=f"h{k}") for k in range(NBLK)]
            c_t = [spool.tile([H, BLK], F32, tag=f"c{k}", name=f"c{k}") for k in range(NBLK)]
            for k in range(NBLK):
                nc.gpsimd.memset(c_t[k][:], 0.0)
                if bt[0] <= k:  # never matmul'd -> h must still be defined
                    nc.gpsimd.memset(h_t[k][:], 0.0)

            # ------------------------------------------------ LSTM main loop
            with (
                tc.tile_pool(name="oh", bufs=2) as ohpool,
                tc.tile_pool(name="gpsum", bufs=2, space="PSUM") as gpsum,
                tc.tile_pool(name="gsb", bufs=3) as gsb,
            ):
                for t in range(L):
                    nb = bt[t]
                    if nb == 0:
                        continue
                    w = nb * BLK
                    oh_sb = ohpool.tile([V + 2, NWORD], BF16, tag="oh", name="oh")
                    nc.sync.dma_start(oh_sb[:, :w], oh_in[t, :, :w])
                    last = t == L - 1
                    for k in range(nb):
                        ps = [gpsum.tile([128, BLK], F32, tag=f"p{m}", name=f"p{m}") for m in range(4)]
                        ohk = oh_sb[:, k * BLK:(k + 1) * BLK]
                        for m in range(4):
                            sl = slice(m * H, (m + 1) * H)
                            if t == 0:
                                nc.tensor.matmul(ps[m][:], lhsT=g65_sb[:, sl],
                                                 rhs=ohk, start=True, stop=True)
                            else:
                                nc.tensor.matmul(ps[m][:], lhsT=g65_sb[:, sl],
                                                 rhs=ohk, start=True, stop=False)
                                nc.tensor.matmul(ps[m][:], lhsT=whht_sb[:, sl],
                                                 rhs=h_t[k][:], start=False, stop=True)
                        ti = gsb.tile([128, BLK], F32, tag="ti", name="ti")
                        tf = gsb.tile([128, BLK], F32, tag="tf", name="tf")
                        tg = gsb.tile([128, BLK], F32, tag="tg", name="tg")
                        nc.scalar.activation(ti[:], ps[0][:], AF.Sigmoid)
                        nc.scalar.activation(tf[:], ps[1][:], AF.Sigmoid)
                        nc.scalar.activation(tg[:], ps[2][:], AF.Tanh)
                        nc.vector.tensor_mul(tg[:], ti[:], tg[:])      # i*g
                        nc.vector.tensor_mul(c_t[k][:], tf[:], c_t[k][:])
                        nc.vector.tensor_add(c_t[k][:], c_t[k][:], tg[:])
                        if not last:
                            to = gsb.tile([128, BLK], F32, tag="to", name="to")
                            tt = gsb.tile([128, BLK], F32, tag="tt", name="tt")
                            nc.scalar.activation(to[:], ps[3][:], AF.Sigmoid)
                            nc.scalar.activation(tt[:], c_t[k][:], AF.Tanh)
                            nc.vector.tensor_mul(h_t[k][:], to[:], tt[:])

            # ------------------------------------------------------- tail
            with (
                tc.tile_pool(name="big", bufs=1) as big,
                tc.tile_pool(name="tpsum", bufs=2, space="PSUM") as tpsum,
                tc.tile_pool(name="cpsum", bufs=2, space="PSUM") as cpsum,
                tc.tile_pool(name="small", bufs=1) as small,
                tc.tile_pool(name="scr", bufs=2) as scrp,
            ):
                cT = big.tile([128, NWORD], F32, tag="cT", name="cT")
                A = big.tile([128, NWORD], F32, tag="A", name="A")
                d_all = small.tile([128, 32], F32, tag="d", name="d")
                s_all = small.tile([128, 32], F32, tag="s", name="s")
                D_all = small.tile([128, NEC * 6], F32, tag="D", name="D")
                cos6 = small.tile([6, PER], F32, tag="cos6", name="cos6")

                for k in range(32):
                    pt = tpsum.tile([128, 128], F32, tag="tp", name="tp")
                    src = c_t[k // 4][:, (k % 4) * 128:(k % 4 + 1) * 128]
                    nc.tensor.transpose(pt[:], src, ident[:])
                    cslc = cT[:, k * 128:(k + 1) * 128]
                    nc.vector.tensor_copy(cslc, pt[:])
                    scr = scrp.tile([128, 128], F32, tag="scr", name="scr")
                    nc.vector.tensor_mul(scr[:], cslc, cslc)
                    nc.vector.tensor_reduce(
                        d_all[:, k:k + 1], scr[:], axis=mybir.AxisListType.X,
                        op=ALU.add)
                nc.vector.tensor_scalar_max(d_all[:], d_all[:], 1e-30)
                nc.scalar.activation(s_all[:], d_all[:], AF.Ln)
                nc.scalar.activation(s_all[:], s_all[:], AF.Exp, scale=-0.5)
                for k in range(32):
                    cslc = cT[:, k * 128:(k + 1) * 128]
                    nc.vector.tensor_scalar_mul(cslc, cslc, s_all[:, k:k + 1])

                nc.sync.dma_start(
                    c_dram[:].rearrange("(k p) h -> p k h", p=128),
                    cT[:].rearrange("p (k h) -> p k h", k=32))

                for b in range(NW * NEC):
                    nc.gpsimd.indirect_dma_start(
                        out=A[:, b * 128:(b + 1) * 128],
                        out_offset=None,
                        in_=c_dram[:],
                        in_offset=bass.IndirectOffsetOnAxis(ap=idx_sb[:, b:b + 1], axis=0),
                    )

                for ec in range(NEC):
                    for k, (i, j) in enumerate(P6):
                        scr = scrp.tile([128, 128], F32, tag="scr", name="scr")
                        nc.vector.tensor_mul(
                            scr[:],
                            A[:, (i * NEC + ec) * 128:(i * NEC + ec + 1) * 128],
                            A[:, (j * NEC + ec) * 128:(j * NEC + ec + 1) * 128])
                        nc.vector.tensor_reduce(
                            D_all[:, ec * 6 + k:ec * 6 + k + 1], scr[:],
                            axis=mybir.AxisListType.X, op=ALU.add)
                for ec in range(NEC):
                    pt = tpsum.tile([128, 128], F32, tag="tp", name="tp")
                    nc.tensor.transpose(pt[:6, :], D_all[:, ec * 6:ec * 6 + 6], ident[:])
                    nc.vector.tensor_copy(cos6[:, ec * 128:(ec + 1) * 128], pt[:6, :])

                r1 = small.tile([36, PER], F32, tag="r1", name="r1")
                r2 = small.tile([32, PER], F32, tag="r2", name="r2")
                e_sb = small.tile([1, PER], F32, tag="e", name="e")
                o_sb = small.tile([1, PER], F32, tag="o", name="o")
                p1 = cpsum.tile([36, PER], F32, tag="cp1", name="cp1")
                for half in range(2):
                    sl = slice(half * 512, (half + 1) * 512)
                    nc.tensor.matmul(p1[:, sl], lhsT=w1_sb[:], rhs=cos6[:, sl],
                                     start=True, stop=True)
                nc.scalar.activation(r1[:], p1[:], AF.Relu, bias=b1_sb[:, 0:1])
                p2 = cpsum.tile([32, PER], F32, tag="cp1", name="cp1")
                for half in range(2):
                    sl = slice(half * 512, (half + 1) * 512)
                    nc.tensor.matmul(p2[:, sl], lhsT=w2_sb[:], rhs=r1[:, sl],
                                     start=True, stop=True)
                nc.scalar.activation(r2[:], p2[:], AF.Relu, bias=b2_sb[:, 0:1])
                p3 = cpsum.tile([1, PER], F32, tag="cp1", name="cp1")
                for half in range(2):
                    sl = slice(half * 512, (half + 1) * 512)
                    nc.tensor.matmul(p3[:, sl], lhsT=wsc_sb[:], rhs=r2[:, sl],
                                     start=True, stop=True)
                nc.scalar.activation(o_sb[:], p3[:], AF.Sigmoid,
                                     bias=bsc_sb[0:1, 0:1])
                nc.sync.dma_start(out_d[:], o_sb[:])

    return nc


_prog_cache = {}


def _get_program(bt):
    key = tuple(int(x) for x in bt)
    if key not in _prog_cache:
        _prog_cache[key] = _build_program(key)
    return _prog_cache[key]


def _run(inputs, trace=False):
    consts = _build_consts(inputs)
    word_ids = np.asarray(inputs["word_ids"])
    lengths = np.asarray(inputs["lengths"])

    preps = []
    for c in range(NCORES):
        sl = slice(c * PER, (c + 1) * PER)
        preps.append(_core_prep(word_ids[sl], lengths[sl]))
    Nt_max = np.stack([p[2] for p in preps]).max(0)
    bt = tuple(int(x) for x in np.ceil(Nt_max / BLK).astype(np.int64))
    widths = [b * BLK for b in bt]

    g65_bf = consts["G65"].astype(ml_dtypes.bfloat16)
    whht_bf = consts["WhhT"].astype(ml_dtypes.bfloat16)
    in_maps = []
    for c in range(NCORES):
        wid_s, lens_s, _, inv = preps[c]
        in_maps.append({
            "oh": _build_onehot(wid_s, lens_s, widths).astype(ml_dtypes.bfloat16),
            "idx": _build_idx(inv),
            "g65": g65_bf, "whht": whht_bf,
            "w1": consts["W1eff"], "b1": consts["b1eff"],
            "w2": consts["W2eff"], "b2": consts["b2eff"],
            "wsc": consts["Wsc"],
            "bsc": np.full((1, 1), consts["bsc"], np.float32),
        })

    nc = _get_program(bt)
    _spill_excess_waits(nc)  # idempotent; HW-compile only (CoreSim dislikes raw NoOps)
    res = run_bass_kernel_spmd(nc, in_maps, list(range(NCORES)), trace=trace)
    out = np.concatenate([np.asarray(r["out"]).reshape(PER) for r in res.results])
    return out.reshape(B, 1).astype(np.float32), res.exec_time_ns


def kernel(**inputs):
    return _run(inputs)[0]
